# revision 2
# baseline (speedup 1.0000x reference)
"""Self-contained Trainium2 Bass kernel for nn_DenseFlashAttention_16123307229343
(GNN segment-softmax message passing). kernel(**inputs) -> np.ndarray.

V3 design: 4-hot interleaved scatter.
- Receivers sharded over 8 cores; host sorts edges by receiver, bins 32
  receivers per chunk (degree-balanced), 4 chunks per quad, T4 128-edge tiles
  per chunk.
- Per edge tile the device builds ONE weighted scatter matrix
  W4AB[e,(ab,slot4)] = (slot4//4 == rslot_e) * exp_weight[e, ab, slot4%4]
  in a single DVE scalar_tensor_tensor (is_equal then mult; 2-byte packed
  operands for the fast DVE mode).
- Scatter matmul: psC[slot4, 0:65] += W4AB^T @ [x[snd] | 1] -- the ones column
  yields the segment-softmax denominators for free.
- Logits via a [65,8] matmul ([x[snd] | len] @ [W_h v_j ; -rds]); exp on ACT.
- Post-scatter normalization per (receiver, head) with per-partition scalars,
  PE transpose, epilogue matmuls against 0.25*W_h@w_out constants, deg-masked
  receiver term (host zero-masks x for empty receivers), residual add.
- Output written feature-major (y_permT [64, slots]); host untransposes.
"""
import sys
sys.path.insert(0, '/opt/trn_rl_repo')

import concourse.mybir as mybir
from concourse.tile import TileContext
from concourse.vector_clock import ScopedClock

MAXW = 1


def _patched_drain_and_barrier(self, tick_clock, wait_clock):
    """(inlined tilefix) the walrus in this container rejects Drain
    instructions carrying >1 sync wait; split the end-of-context drain."""
    nc = self.nc
    drain_inst = nc.sync.drain()
    wait_clock.add_sem_waits(drain_inst.ins, ScopedClock({None: tick_clock.global_clock}))
    si = drain_inst.ins.sync_info
    waits = list(si.on_wait) if si is not None else []
    if len(waits) > MAXW:
        si.on_wait = waits[:MAXW]
        rest = waits[MAXW:]
        for i in range(0, len(rest), MAXW):
            d2 = nc.sync.drain()
            d2.ins.sync_info = mybir.SyncInfo(on_wait=rest[i:i + MAXW], on_update=[])
    nc.all_engine_barrier()
    popped = nc._tile_sem_poison_stack.pop()
    assert popped is self._sem_poison
    nc.clear_and_free_semaphores(list(self.sems.allocated().values()))
    nc.all_engine_barrier()


def install():
    TileContext._drain_and_barrier = _patched_drain_and_barrier


_ctr = [0]


def split_sync_waits(nc, maxw=1):
    """Hoist extra sync waits onto no-op carriers (walrus 1-wait limit)."""
    for f in nc.m.functions:
        for blk in f.blocks:
            lst = blk.instructions
            i = 0
            while i < len(lst):
                ins = lst[i]
                si = ins.sync_info
                if si is None:
                    i += 1
                    continue
                waits = list(si.on_wait)
                if len(waits) <= maxw:
                    i += 1
                    continue
                si.on_wait = waits[-maxw:]
                rest = waits[:-maxw]
                carriers = []
                for j in range(0, len(rest), maxw):
                    _ctr[0] += 1
                    nop = mybir.InstEventSemaphore(name=f"waitnop_{_ctr[0]}", ins=[], outs=[])
                    nop.engine = ins.engine
                    nop.sync_info = mybir.SyncInfo(on_wait=rest[j:j + maxw],
                                                   on_update=[])
                    nc.register_instruction(nop, overwrite=True)
                    carriers.append(nop)
                for k, nop in enumerate(carriers):
                    lst.insert(i + k, nop)
                i += len(carriers) + 1


import math
import numpy as np
import ml_dtypes

import concourse.bass as bass

bf16 = ml_dtypes.bfloat16
FP32 = mybir.dt.float32
BF16 = mybir.dt.bfloat16
ALU = mybir.AluOpType
ACTF = mybir.ActivationFunctionType

RC = 32          # receivers per chunk (4-hot: 32 * 4 heads = 128 slots)
QC = 4           # chunks per quad


class Params:
    def __init__(self, N, E, F=64, H=4, NC=8, G=None):
        self.N, self.E, self.F, self.H, self.NC = N, E, F, H, NC
        self.RPC = N // NC                      # receivers per core
        ch = math.ceil(self.RPC / RC)
        self.CH = math.ceil(ch / QC) * QC       # chunks per core (quad-aligned)
        self.QUADS = self.CH // QC
        self.SLOTS = self.CH * RC               # receiver slots per core


def host_prep(p: Params, x, edge_index, edge_len):
    N, NC = p.N, p.NC
    snd = np.asarray(edge_index[0]).astype(np.int64)
    rcv = np.asarray(edge_index[1]).astype(np.int64)
    deg = np.bincount(rcv, minlength=N)
    order = np.argsort(rcv, kind='stable')
    starts = np.zeros(N + 1, np.int64)
    np.cumsum(deg, out=starts[1:])

    import heapq

    def lpt(idxs_desc, dloc, nbins, cap, binoff, bin_of, slot_of):
        """LPT-assign receivers (desc degree) to least-loaded bin with <RC
        receivers; returns max bin load."""
        if nbins == 0:
            assert len(idxs_desc) == 0
            return 0
        h = [(0, i, 0) for i in range(nbins)]
        heapq.heapify(h)
        deferred = []
        mx = 0
        for r in idxs_desc:
            dr = int(dloc[r])
            while True:
                load, i, cnt = heapq.heappop(h)
                if cnt < RC:
                    bin_of[r] = binoff + i
                    slot_of[r] = cnt
                    heapq.heappush(h, (load + dr, i, cnt + 1))
                    mx = max(mx, load + dr)
                    break
                deferred.append((load, i, cnt))
            if deferred:
                for it in deferred:
                    heapq.heappush(h, it)
                deferred = []
        return mx

    # K2 = count of 2-tile (cap 256) chunks holding the smallest-degree
    # receivers; the rest are 3-tile (cap 384). Max feasible K2 across cores.
    scratch_b = np.empty(p.RPC, np.int64)
    scratch_s = np.empty(p.RPC, np.int64)
    maxE = max(int(deg[k * p.RPC:(k + 1) * p.RPC].sum()) for k in range(NC))
    ub = min(p.CH, p.RPC // RC, (384 * p.CH - maxE) // 128)
    ub = max(0, (ub // 4) * 4)
    K2 = 0
    for k2try in range(ub, -4, -4):
        ok = True
        for k in range(NC):
            dloc = deg[k * p.RPC:(k + 1) * p.RPC]
            asc = np.argsort(dloc, kind='stable')
            n2 = RC * k2try
            if k2try and lpt(asc[:n2][::-1], dloc, k2try, 256, 0,
                             scratch_b, scratch_s) > 256:
                ok = False
                break
            if lpt(asc[n2:][::-1], dloc, p.CH - k2try, 384, k2try,
                   scratch_b, scratch_s) > 384:
                ok = False
                break
        if ok:
            K2 = k2try
            break
    assert K2 % 4 == 0
    cores = []
    for k in range(NC):
        dloc = deg[k * p.RPC:(k + 1) * p.RPC]
        asc = np.argsort(dloc, kind='stable')
        bin_of = np.empty(p.RPC, np.int64)
        slot_of = np.empty(p.RPC, np.int64)
        n2 = RC * K2
        lpt(asc[:n2][::-1], dloc, K2, 256, 0, bin_of, slot_of)
        lpt(asc[n2:][::-1], dloc, p.CH - K2, 384, K2, bin_of, slot_of)
        cores.append(dict(bin_of=bin_of, slot_of=slot_of))
    T4 = K2    # meta['T'] carries K2 (cache key + build_program arg)
    tcnt = np.where(np.arange(p.CH) < K2, 2, 3)
    toff = np.zeros(p.CH + 1, np.int64)
    np.cumsum(tcnt, out=toff[1:])

    xf = np.asarray(x, np.float32)
    el = np.asarray(edge_len, np.float32)
    NT = int(toff[-1])
    per_core = []
    for k in range(NC):
        c = cores[k]
        lo = k * p.RPC
        esnd = np.full((NT * 128,), -1, np.int64)
        lens = np.zeros((NT * 128,), np.float32)
        ro = np.full((NT * 128,), 255.0, np.float32)
        fill = np.zeros(p.CH, np.int64)
        for r_local in np.argsort(c['bin_of'], kind='stable'):
            b = c['bin_of'][r_local]
            n = lo + r_local
            e0, e1 = starts[n], starts[n + 1]
            cnt = e1 - e0
            if cnt == 0:
                continue
            base = toff[b] * 128 + fill[b]
            eidx = order[e0:e1]
            esnd[base:base + cnt] = snd[eidx]
            lens[base:base + cnt] = el[eidx]
            ro[base:base + cnt] = c['slot_of'][r_local]
            fill[b] += cnt
        assert (fill <= tcnt * 128).all()

        real = esnd >= 0
        # xe_aug [128, NT, 65]: gathered sender features + ones column
        xe = np.zeros((NT * 128, p.F + 1), np.float32)
        xe[real, 0:p.F] = xf[esnd[real]]
        xe[:, p.F] = 1.0
        xe_aug = np.ascontiguousarray(
            xe.reshape(NT, 128, p.F + 1).transpose(1, 0, 2)).astype(bf16)
        # xeT65 [65, NT*128]: same features transposed + edge_len row
        xt = np.zeros((p.F + 1, NT * 128), np.float32)
        xt[0:p.F, real] = xf[esnd[real]].T
        xt[p.F, :] = lens
        xeT65 = xt.astype(bf16)
        roT = np.ascontiguousarray(
            ro.reshape(NT, 128).T).astype(bf16)
        # receiver features at their slots (transposed), deg-masked + raw
        slot_global = c['bin_of'] * RC + c['slot_of']
        xr = np.zeros((p.SLOTS, p.F), np.float32)
        xr[slot_global] = xf[lo:lo + p.RPC]
        x_rcvT_r = np.ascontiguousarray(xr.T)                       # fp32
        mask = (deg[lo:lo + p.RPC] > 0).astype(np.float32)
        xrm = np.zeros((p.SLOTS, p.F), np.float32)
        xrm[slot_global] = xf[lo:lo + p.RPC] * mask[:, None]
        xrm2 = np.concatenate([xrm.T, xrm.T], axis=0)               # [128, SLOTS]
        x_rcvT_m2 = np.ascontiguousarray(xrm2).astype(bf16)
        per_core.append(dict(xe_aug=xe_aug, xeT65=xeT65, roT=roT,
                             x_rcvT_r=x_rcvT_r, x_rcvT_m2=x_rcvT_m2,
                             slot_global=slot_global))
    return dict(T=T4, per_core=per_core)


def build_program(p: Params, K2: int):
    nc = bass.Bass("TRN2", target_bir_lowering=False, debug=False,
                   num_devices=p.NC)
    F, H = p.F, p.H
    NT = 2 * K2 + 3 * (p.CH - K2)
    QL = K2 // 4                 # light quads (2 tiles/chunk)
    QT = 12                      # max tiles per quad (buffer shape)
    S = p.SLOTS

    xe_aug = nc.dram_tensor("xe_aug", [128, NT, F + 1], BF16, kind="ExternalInput").ap()
    xeT65 = nc.dram_tensor("xeT65", [F + 1, NT * 128], BF16, kind="ExternalInput").ap()
    roT = nc.dram_tensor("roT", [128, NT], BF16, kind="ExternalInput").ap()
    wvl_in = nc.dram_tensor("wvl", [F + 1, 2 * H], BF16, kind="ExternalInput").ap()
    wo2_in = nc.dram_tensor("wo2", [128, H, F], BF16, kind="ExternalInput").ap()
    wm2o_in = nc.dram_tensor("wm2o", [128, F], BF16, kind="ExternalInput").ap()
    iota4_in = nc.dram_tensor("iota4", [128, RC, 4], BF16, kind="ExternalInput").ap()
    ident_in = nc.dram_tensor("ident", [128, 128], BF16, kind="ExternalInput").ap()
    xrm2_in = nc.dram_tensor("x_rcvT_m2", [128, S], BF16, kind="ExternalInput").ap()
    xrr_in = nc.dram_tensor("x_rcvT_r", [F, S], FP32, kind="ExternalInput").ap()
    y_permT = nc.dram_tensor("y_permT", [F, S], FP32, kind="ExternalOutput").ap()

    with TileContext(nc) as tc:
        import contextlib
        ctx = contextlib.ExitStack()
        with ctx:
            const = ctx.enter_context(tc.tile_pool(name="const", bufs=1))
            wvl_s = const.tile([F + 1, 2 * H], BF16)
            nc.sync.dma_start(out=wvl_s[:], in_=wvl_in[:])
            wo2_s = const.tile([128, H, F], BF16)
            nc.sync.dma_start(out=wo2_s[:], in_=wo2_in[:])
            wm2o_s = const.tile([128, F], BF16)
            nc.sync.dma_start(out=wm2o_s[:], in_=wm2o_in[:])
            iota4_s = const.tile([128, RC, 4], BF16)
            nc.sync.dma_start(out=iota4_s[:], in_=iota4_in[:])
            ident_s = const.tile([128, 128], BF16)
            nc.sync.dma_start(out=ident_s[:], in_=ident_in[:])
            xrm2_s = const.tile([128, S], BF16)
            nc.sync.dma_start(out=xrm2_s[:], in_=xrm2_in[:])
            xrr_s = const.tile([F, S], FP32)
            nc.sync.dma_start(out=xrr_s[:], in_=xrr_in[:])

            with tc.tile_pool(name="edge", bufs=2) as ep, \
                 tc.tile_pool(name="w4", bufs=2) as wp, \
                 tc.tile_pool(name="ret", bufs=2) as rp, \
                 tc.tile_pool(name="psl", bufs=1, space="PSUM") as pslp, \
                 tc.tile_pool(name="cps", bufs=2, space="PSUM") as cps, \
                 tc.tile_pool(name="tps", bufs=1, space="PSUM") as tps, \
                 tc.tile_pool(name="yps", bufs=2, space="PSUM") as yps:
                for q in range(p.QUADS):
                    Tq = 2 if q < QL else 3          # tiles per chunk
                    nq = QC * Tq                     # tiles this quad
                    t0 = 8 * q if q < QL else 2 * K2 + 12 * (q - QL)
                    xe_s = ep.tile([128, QT, F + 1], BF16, tag="xe")
                    nc.sync.dma_start(out=xe_s[:, 0:nq, :],
                                      in_=xe_aug[:, t0:t0 + nq, :])
                    xt_s = ep.tile([F + 1, QT * 128], BF16, tag="xt")
                    nc.sync.dma_start(out=xt_s[:, 0:nq * 128],
                                      in_=xeT65[:, t0 * 128:(t0 + nq) * 128])
                    ro_s = ep.tile([128, QT], BF16, tag="ro")
                    nc.sync.dma_start(out=ro_s[:, 0:nq], in_=roT[:, t0:t0 + nq])

                    # logits for the whole quad -> one PSUM bank
                    psL = pslp.tile([128, QT, 2, H], FP32, space="PSUM", tag="psL")
                    for t in range(nq):
                        nc.tensor.matmul(out=psL[:, t, :, :],
                                         lhsT=xt_s[:, t * 128:(t + 1) * 128],
                                         rhs=wvl_s[:],
                                         start=True, stop=True)
                    wts_s = ep.tile([128, QT, 2, H], BF16, tag="wts")
                    nc.scalar.activation(out=wts_s[:, 0:nq, :, :],
                                         in_=psL[:, 0:nq, :, :], func=ACTF.Exp)

                    # weighted 4-hot scatter matrices, one DVE op per tile/half
                    w4_s = wp.tile([128, QT, 2, RC, 4], BF16, tag="w4")
                    for t in range(nq):
                        for ab in range(2):
                            nc.vector.scalar_tensor_tensor(
                                out=w4_s[:, t, ab, :, :],
                                in0=iota4_s[:],
                                scalar=ro_s[:, t:t + 1],
                                in1=wts_s[:, t, ab, None, :].to_broadcast(
                                    [128, RC, 4]),
                                op0=ALU.is_equal, op1=ALU.mult)

                    # scatter: per chunk accumulate A/B over its Tq tiles
                    psA = cps.tile([128, QC, F + 1], FP32, space="PSUM", tag="psA")
                    psB = cps.tile([128, QC, F + 1], FP32, space="PSUM", tag="psB")
                    for c in range(QC):
                        for k in range(Tq):
                            t = c * Tq + k
                            nc.tensor.matmul(out=psA[:, c, :],
                                             lhsT=w4_s[:, t, 0, :, :],
                                             rhs=xe_s[:, t, :],
                                             start=(k == 0), stop=(k == Tq - 1))
                            nc.tensor.matmul(out=psB[:, c, :],
                                             lhsT=w4_s[:, t, 1, :, :],
                                             rhs=xe_s[:, t, :],
                                             start=(k == 0), stop=(k == Tq - 1))

                    # rcp = 1/max(den, eps)   (0.25 folded into Wo constants)
                    rcp_s = rp.tile([128, 2, QC], FP32, tag="rcp")
                    nc.vector.tensor_scalar(out=rcp_s[:, 0, :], in0=psA[:, :, F],
                                            scalar1=1e-30, scalar2=None,
                                            op0=ALU.max)
                    nc.vector.tensor_scalar(out=rcp_s[:, 1, :], in0=psB[:, :, F],
                                            scalar1=1e-30, scalar2=None,
                                            op0=ALU.max)
                    nc.vector.reciprocal(out=rcp_s[:], in_=rcp_s[:])

                    # u4 = psA*rcpA + psB*rcpB  -> bf16
                    tmpA = rp.tile([128, QC, F], BF16, tag="tmpA")
                    tmpB = rp.tile([128, QC, F], BF16, tag="tmpB")
                    nc.vector.tensor_tensor(
                        out=tmpA[:], in0=psA[:, :, 0:F],
                        in1=rcp_s[:, 0, :, None].to_broadcast([128, QC, F]),
                        op=ALU.mult)
                    nc.vector.tensor_tensor(
                        out=tmpB[:], in0=psB[:, :, 0:F],
                        in1=rcp_s[:, 1, :, None].to_broadcast([128, QC, F]),
                        op=ALU.mult)
                    u4_s = rp.tile([128, QC, F], BF16, tag="u4")
                    nc.vector.tensor_tensor(out=u4_s[:], in0=tmpA[:], in1=tmpB[:],
                                            op=ALU.add)

                    # per-chunk transpose [128 slot4, 64f] -> [64f, 128 slot4],
                    # then de-interleave heads (slot4 = r*4+h -> [h, r]) via a
                    # free-dim-permuted copy. All operands base-partition 0,
                    # no strided matmul APs (this walrus mislowers both).
                    u4T_s = rp.tile([F, QC, H, RC], BF16, tag="u4T")
                    for c in range(QC):
                        psT = tps.tile([F, RC, H], BF16, space="PSUM", tag="psT")
                        nc.tensor.transpose(out=psT[:],
                                            in_=u4_s[:, c, :],
                                            identity=ident_s[:])
                        if c % 2 == 0:
                            nc.scalar.copy(out=u4T_s[:, c, :, :],
                                           in_=psT[:].transpose([0, 2, 1]))
                        else:
                            nc.vector.tensor_copy(out=u4T_s[:, c, :, :],
                                                  in_=psT[:].transpose([0, 2, 1]))

                    # epilogue: psY[g, slot_r] = sum_h (0.25 W_h wout)^T u_h^T
                    #           + (-0.5 sum W_h wout)^T (deg-masked x_r)^T
                    psY = yps.tile([F, QC * RC], FP32, space="PSUM", tag="psY")
                    for h in range(H):
                        nc.tensor.matmul(
                            out=psY[:],
                            lhsT=wo2_s[0:64, h, :],
                            rhs=u4T_s[:, :, h, :],
                            start=(h == 0), stop=False)
                    nc.tensor.matmul(
                        out=psY[:],
                        lhsT=wm2o_s[0:64, :],
                        rhs=xrm2_s[0:64, q * 128:(q + 1) * 128],
                        start=False, stop=True)

                    ybufT = rp.tile([F, QC * RC], FP32, tag="ybufT")
                    nc.vector.tensor_tensor(
                        out=ybufT[:], in0=psY[:],
                        in1=xrr_s[:, q * 128:(q + 1) * 128], op=ALU.add)
                    nc.sync.dma_start(out=y_permT[:, q * 128:(q + 1) * 128],
                                      in_=ybufT[:])
    split_sync_waits(nc, maxw=1)
    nc.finalize()
    return nc


def make_in_maps(p: Params, meta, x, w_proj, rs, ts, rds, w_out):
    H, F = p.H, p.F
    wp = np.asarray(w_proj, np.float64)
    rsd = np.asarray(rs, np.float64)
    tsd = np.asarray(ts, np.float64)
    wo = np.asarray(w_out, np.float64)
    rdsf = float(rds)

    wvl = np.zeros((F + 1, 2 * H), np.float64)
    for h in range(H):
        wvl[0:F, h] = wp[h] @ rsd[h]          # radial (ab=0)
        wvl[0:F, H + h] = wp[h] @ tsd[h]      # tangential (ab=1)
    wvl[F, 0:H] = -rdsf
    wvl_b = wvl.astype(bf16)

    wo2 = np.zeros((128, H, F), np.float64)
    for h in range(H):
        blk = 0.25 * (wp[h] @ wo)
        wo2[0:64, h, :] = blk
        wo2[64:128, h, :] = blk
    wo2_b = wo2.astype(bf16)

    wm = -0.5 * (wp.sum(axis=0) @ wo)
    wm2o = np.concatenate([wm, wm], axis=0).astype(bf16)   # [128, F]

    iota4 = np.tile(np.arange(RC, dtype=np.float32)[:, None], (1, 4))
    iota4_b = np.broadcast_to(iota4[None, :, :], (128, RC, 4)).astype(bf16)
    iota4_b = np.ascontiguousarray(iota4_b)
    ident_b = np.eye(128, dtype=np.float32).astype(bf16)

    in_maps = []
    for k in range(p.NC):
        c = meta['per_core'][k]
        in_maps.append({
            "xe_aug": c['xe_aug'], "xeT65": c['xeT65'], "roT": c['roT'],
            "wvl": wvl_b, "wo2": wo2_b, "wm2o": wm2o,
            "iota4": iota4_b, "ident": ident_b,
            "x_rcvT_m2": c['x_rcvT_m2'], "x_rcvT_r": c['x_rcvT_r'],
        })
    return in_maps


def assemble(p: Params, meta, results):
    y = np.zeros((p.N, p.F), np.float32)
    for k in range(p.NC):
        c = meta['per_core'][k]
        yT = results[k]["y_permT"]            # [F, SLOTS]
        y[k * p.RPC:(k + 1) * p.RPC] = yT[:, c['slot_global']].T
    return y


install()

_CACHE = {}


def kernel(x, edge_index, edge_vec, edge_len, w_proj, radial_score,
           tangential_score, radial_distance_scale, w_out):
    x = np.asarray(x, np.float32)
    edge_index = np.asarray(edge_index)
    edge_len = np.asarray(edge_len, np.float32)
    w_proj = np.asarray(w_proj, np.float32)
    rs = np.asarray(radial_score, np.float32)
    ts = np.asarray(tangential_score, np.float32)
    rds = np.float32(np.asarray(radial_distance_scale))
    w_out_ = np.asarray(w_out, np.float32)

    N, F = x.shape
    H = w_proj.shape[0]
    E = edge_index.shape[1]
    p = Params(N, E, F=F, H=H, NC=8)
    meta = host_prep(p, x, edge_index, edge_len)
    T4 = meta['T']
    key = (N, E, F, H, T4)
    if key not in _CACHE:
        _CACHE[key] = build_program(p, T4)
    nc = _CACHE[key]
    in_maps = make_in_maps(p, meta, x, w_proj, rs, ts, rds, w_out_)
    from concourse.bass_utils import run_bass_kernel_spmd
    res = run_bass_kernel_spmd(nc, in_maps, list(range(p.NC)))
    y = assemble(p, meta, [res.results[i] for i in range(p.NC)])
    return y.astype(np.float32)
